# revision 1
# baseline (speedup 1.0000x reference)
"""Trainium2 Bass kernel for nn_Gate_Net (Toeplitz + hard-sigmoid prob + cumprod gate).

Reference computation (per document row of 1024 scores):
  s = doc[1:-1]                      # n = 1022
  score_hat[r, j] = s[j-1-r] if j-1-r >= 0 else 0      # [1021, 1022]
  p[r, j] = clamp(10*(score_hat - s[j]) + 1, 0, 1)      # hard branch, res=0.1
  fwd = cumprod(p, axis=0); bwd = same with s reversed
  out = stack([fwd, bwd]) per doc -> full [32, 2, 1021, 1022] f32

Device algorithm (per doc-direction, column-major):
  Column j of the output is a cumprod over m of factors
    q(j, m) = clamp(10*s[j-1-m] + c_j, 0, 1), c_j = 1 - 10*s[j]   (m < j)
    q(j, m) = clamp(c_j, 0, 1)                                     (m >= j)
  We materialize, with ONE diagonal-AP DMA per doc-dir, the sheared tile
    B[p, t] = arr[127 - p + t]   where arr = [0, reversed(10*s), 0-pad]
  so that every 128-column block's factor matrix is a plain uniform-offset
  slice of B (zeros beyond the data edge give exactly the boundary factor).
  Then: tensor_scalar(add c_j, min 1) -> Relu -> tensor_tensor_scan
  (cumprod along the free axis) -> PE transpose -> contiguous row stores.

Sharding: pure data parallel, 8 doc-dirs per core (4 docs x fwd/bwd).
"""
import numpy as np

import concourse.bass as bass
import concourse.bacc as bacc
import concourse.tile as tile
from concourse import mybir
from concourse import bass_utils
from concourse.masks import make_identity

P = 128
N = 1022          # columns j per doc-dir
ROWS = N - 1      # 1021 output rows
NB = 8            # column blocks (last has 126 valid columns)
MB = 8            # row blocks (last has 125 valid rows)
ARRW = 1152       # padded diag-source array width
BW = 1024         # sheared tile width

_NC_CACHE: dict = {}


def build_nc(n_dd: int = 8):
    """Build the single-core Bass program processing n_dd doc-dirs."""
    nc = bacc.Bacc("TRN2", target_bir_lowering=False, debug=False, num_devices=8)
    arr = nc.dram_tensor("arr", [n_dd, ARRW], mybir.dt.float32, kind="ExternalInput")
    cc = nc.dram_tensor("cc", [n_dd, P, 16], mybir.dt.float32, kind="ExternalInput")
    out = nc.dram_tensor("out", [n_dd, ROWS, N], mybir.dt.float32, kind="ExternalOutput")

    add_op = mybir.AluOpType.add
    min_op = mybir.AluOpType.min
    mult_op = mybir.AluOpType.mult
    relu = mybir.ActivationFunctionType.Relu

    with tile.TileContext(nc) as tc:
        with (
            tc.tile_pool(name="consts", bufs=1) as consts,
            tc.tile_pool(name="bsrc", bufs=2) as bsrc_pool,
            tc.tile_pool(name="qpool", bufs=2) as qpool,
            tc.tile_pool(name="rpool", bufs=2) as rpool,
            tc.tile_pool(name="cpool", bufs=2) as cpool,
            tc.tile_pool(name="outp", bufs=2) as outp,
            tc.tile_pool(name="psum", bufs=8, space="PSUM") as psum,
        ):
            # flip permutation: flip[k, n] = 1 iff k + n == P-1.  Used as the
            # matmul rhs in the PE transpose so that the partition-flipped
            # column order (p <-> j = jb*128 + 127 - p) comes out natural.
            flip = consts.tile([P, P], mybir.dt.float32)
            nc.gpsimd.memset(flip[:], 0.0)
            nc.gpsimd.affine_select(
                out=flip[:], in_=flip[:],
                compare_op=mybir.AluOpType.not_equal, fill=1.0,
                base=-(P - 1), pattern=[[1, P]], channel_multiplier=1,
            )
            zeros = consts.tile([P, ROWS], mybir.dt.float32)
            nc.vector.memset(zeros[:], 0.0)

            for dd in range(n_dd):
                B = bsrc_pool.tile([P, BW], mybir.dt.float32, tag="B")
                diag_src = bass.AP(
                    tensor=arr, offset=dd * ARRW, ap=[[1, P], [1, BW]]
                )
                nc.sync.dma_start(out=B[:], in_=diag_src)

                csb = cpool.tile([P, 16], mybir.dt.float32, tag="csb")
                nc.sync.dma_start(out=csb[:], in_=cc[dd, :, :])

                outsb = [
                    outp.tile([P, N], mybir.dt.float32, tag=f"o{mb}", name=f"osb{mb}")
                    for mb in range(MB)
                ]

                for jb in range(NB):
                    W = min(jb * 128 + 128, ROWS)
                    y = 896 - jb * 128
                    Q = qpool.tile([P, ROWS], mybir.dt.float32, tag="Q", name="Q")
                    # q_pre = min(B_slice + c_j, 1); factor = relu(q_pre)
                    nc.vector.tensor_scalar(
                        out=Q[:, 0:W],
                        in0=B[:, y:y + W],
                        scalar1=csb[:, jb:jb + 1],
                        scalar2=1.0,
                        op0=add_op,
                        op1=min_op,
                    )
                    nc.scalar.activation(
                        out=Q[:, 0:W], in_=Q[:, 0:W], func=relu, bias=0.0, scale=1.0
                    )
                    if W < ROWS:
                        # tail factors: clamp(c_j, 0, 1) = relu(min(c_j, 1))
                        nc.scalar.activation(
                            out=Q[:, W:ROWS],
                            in_=B[:, 0:ROWS - W],
                            func=relu,
                            bias=csb[:, 8 + jb:8 + jb + 1],
                            scale=0.0,
                        )
                    R = rpool.tile([P, ROWS], mybir.dt.float32, tag="R", name="R")
                    nc.vector.tensor_tensor_scan(
                        out=R[:],
                        data0=Q[:],
                        data1=zeros[:],
                        initial=1.0,
                        op0=mult_op,
                        op1=add_op,
                    )
                    cols = 126 if jb == NB - 1 else 128
                    for mb in range(MB):
                        chunk = 125 if mb == MB - 1 else 128
                        pt = psum.tile([P, P], mybir.dt.float32, tag="pt", name="pt")
                        nc.tensor.transpose(
                            pt[:chunk, :], R[:, mb * 128:mb * 128 + chunk], flip[:]
                        )
                        dst = outsb[mb][:chunk, jb * 128:jb * 128 + cols]
                        if mb % 2 == 0:
                            nc.scalar.copy(out=dst, in_=pt[:chunk, 0:cols])
                        else:
                            nc.vector.tensor_copy(dst, pt[:chunk, 0:cols])

                for mb in range(MB):
                    chunk = 125 if mb == MB - 1 else 128
                    nc.sync.dma_start(
                        out=out[dd, mb * 128:mb * 128 + chunk, :],
                        in_=outsb[mb][:chunk, :],
                    )
    nc.compile()
    return nc


def get_nc(n_dd: int = 8):
    if n_dd not in _NC_CACHE:
        _NC_CACHE[n_dd] = build_nc(n_dd)
    return _NC_CACHE[n_dd]


def make_core_inputs(docs_core: np.ndarray) -> dict:
    """docs_core: [n_docs, 1024] f32 -> in_map with arr/cc for n_docs*2 doc-dirs."""
    n_docs = docs_core.shape[0]
    n_dd = n_docs * 2
    arr = np.zeros((n_dd, ARRW), np.float32)
    cc = np.zeros((n_dd, P, 16), np.float32)
    for dl in range(n_docs):
        s = docs_core[dl, 1:-1].astype(np.float32)  # 1022
        for t in range(2):
            v = s if t == 0 else s[::-1]
            dd = dl * 2 + t
            v10 = (np.float32(10.0) * v).astype(np.float32)
            arr[dd, 1:1 + N] = v10[::-1]
            cvals = (np.float32(1.0) - v10).astype(np.float32)
            # partition p holds column j = jb*128 + (127 - p)
            for jb in range(NB):
                seg = cvals[jb * 128: jb * 128 + 128]
                cseg = np.zeros(P, np.float32)
                cseg[P - len(seg):] = seg[::-1]
                cc[dd, :, jb] = cseg
                cc[dd, :, 8 + jb] = np.minimum(cseg, np.float32(1.0))
    return {"arr": arr, "cc": cc}


def kernel(score: np.ndarray, score_idx: np.ndarray) -> np.ndarray:
    score = np.asarray(score, dtype=np.float32)
    score_idx = np.asarray(score_idx)
    docs = score[score_idx]  # [B, L] gather
    Bn, L = docs.shape       # 32, 1024
    n_cores = 8
    docs_per_core = Bn // n_cores  # 4

    in_maps = [
        make_core_inputs(docs[c * docs_per_core:(c + 1) * docs_per_core])
        for c in range(n_cores)
    ]
    nc = get_nc(docs_per_core * 2)
    res = bass_utils.run_bass_kernel_spmd(nc, in_maps, core_ids=list(range(n_cores)))
    full = np.empty((Bn, 2, ROWS, N), np.float32)
    for c in range(n_cores):
        o = np.asarray(res.results[c]["out"]).reshape(docs_per_core * 2, ROWS, N)
        for dl in range(docs_per_core):
            for t in range(2):
                full[c * docs_per_core + dl, t] = o[dl * 2 + t]
    return full



# revision 6
# speedup vs baseline: 7.0235x; 7.0235x over previous
"""Trainium2 Bass kernel for nn_Gate_Net (Toeplitz + hard-sigmoid prob + cumprod gate).

Reference (per document row of 1024 scores):
  s = doc[1:-1]                                  # n = 1022
  hat[m, j] = s[j-1-m] if j-1-m >= 0 else 0      # [1021, 1022]
  p[m, j]  = clamp(10*(hat - s[j]) + 1, 0, 1)    # hard branch, res = 0.1
  fwd = cumprod(p, axis=0); bwd = same with s reversed
  out = stack([fwd, bwd]) per doc -> full [32, 2, 1021, 1022] f32

Key structure: with v = 10*s and c_j = 1 - v_j, factor(j, m) =
clamp(v[j-1-m] + c_j, 0, 1) (v[<0] := 0 reproduces the boundary rule).
A column's cumprod hits EXACT 0 at the first m with v[j-1-m] + c_j <= 0,
and everything below stays 0.  On real inputs ~99% of columns die within
the first K=128 rows, so:

  1. Prefix pass (device): rows 0..K-1 for all (padded) 1024 columns of
     all 8 doc-dirs at once.  Partition p = (dd, col-block-of-64); free
     axis t = j'*K + m.  q built from a shifted AP over a per-partition
     slice of v plus a broadcast c, clamped, then ONE segmented
     tensor_tensor_scan (scan: state = data0*state + data1; at each
     column start data0=0/data1=q0 resets the chain).  Result is DMAd
     with 128 contiguous 32 KiB descriptors -- no transpose needed; the
     host reorders (col-major -> row-major) on 4 MiB/core.
  2. Survivor pass (device): columns with no exact-zero factor among
     rows < K (found host-side with a sliding-window min; ~130/core)
     are scanned at full length col-major and the host scatters
     rows K.. into the output.
  3. Everything else is exactly 0 and is never written (host assembles
     into np.zeros).

Sharding: pure data parallel, 4 docs (8 doc-dirs) per core.
"""
import numpy as np

import concourse.bass as bass
import concourse.bacc as bacc
import concourse.tile as tile
from concourse import mybir
from concourse import bass_utils

P = 128            # SBUF partitions
L = 1024           # sentences per document
N = L - 2          # 1022 real columns per doc-dir
ROWS = N - 1       # 1021 output rows
K = 128            # dense prefix rows computed for every column
NCOL = 1024        # padded column count (cols N..NCOL-1 are garbage)
CJ = NCOL // 16    # 64 columns per partition slot
FREE = CJ * K      # 8192 free elems per partition in the prefix pass
ARRW = 1280        # padded v-array width: [K zeros][1022 v][pad]
SURV_ROWS = ROWS - K   # 893 rows written per survivor column
SCAN_CHUNKS = 2    # prefix scan/store chunks (column-aligned, carry-free)

_NC_CACHE: dict = {}


def _ap(t: bass.AP, delta: int, dims):
    """Custom free-dim AP over tile t (keeps t's partition pair)."""
    return bass.AP(tensor=t.tensor, offset=t.offset + delta,
                   ap=[list(t.ap[0])] + [list(d) for d in dims])


def build_nc(n_dd: int, surv_tiles: tuple):
    """Bass program: prefix pass for n_dd=8 doc-dirs + survivor scans."""
    assert n_dd == 8
    nc = bacc.Bacc("TRN2", target_bir_lowering=False, debug=False, num_devices=8)
    arr = nc.dram_tensor("arr", [n_dd, ARRW], mybir.dt.float32, kind="ExternalInput")
    cap = sum(surv_tiles)
    if cap:
        sc = nc.dram_tensor("sc", [cap, L], mybir.dt.float32, kind="ExternalInput")
        s1 = nc.dram_tensor("s1", [cap, SURV_ROWS], mybir.dt.float32,
                            kind="ExternalOutput")
    s0 = nc.dram_tensor("s0", [P, FREE], mybir.dt.float32, kind="ExternalOutput")

    add = mybir.AluOpType.add
    mult = mybir.AluOpType.mult
    amin = mybir.AluOpType.min
    amax = mybir.AluOpType.max

    with tile.TileContext(nc) as tc:
        with (
            tc.tile_pool(name="io", bufs=1) as io,
            tc.tile_pool(name="work", bufs=1) as work,
        ):
            # ---- prefix pass -------------------------------------------------
            # arr_sb[p, t] = v[J0 + t - K], J0 = (p % 16) * 64   (p = dd*16 + slot)
            arr_sb = io.tile([P, K + CJ], mybir.dt.float32)
            nc.sync.dma_start(
                out=arr_sb[:],
                in_=bass.AP(tensor=arr, offset=0,
                            ap=[[ARRW, 8], [CJ, 16], [1, K + CJ]]),
            )
            # c[p, j'] = 1 - v[J0 + j']
            c_sb = io.tile([P, CJ], mybir.dt.float32)
            nc.vector.tensor_scalar(
                out=c_sb[:], in0=arr_sb[:, K:K + CJ],
                scalar1=-1.0, scalar2=1.0, op0=mult, op1=add,
            )
            q = work.tile([P, FREE], mybir.dt.float32)
            d1 = work.tile([P, FREE], mybir.dt.float32)
            R = work.tile([P, FREE], mybir.dt.float32)
            nc.gpsimd.memset(d1[:], 0.0)
            # q[p, j'*K + m] = v[J0 + j' - 1 - m] + c[J0 + j']
            nc.vector.tensor_tensor(
                out=_ap(q, 0, [[K, CJ], [1, K]]),
                in0=_ap(arr_sb, K - 1, [[1, CJ], [-1, K]]),
                in1=_ap(c_sb, 0, [[1, CJ], [0, K]]),
                op=add,
            )
            nc.vector.tensor_scalar(
                out=q[:], in0=q[:], scalar1=1.0, scalar2=0.0, op0=amin, op1=amax,
            )
            # segment resets at m == 0: d1 takes q's value, q becomes 0
            nc.scalar.copy(out=_ap(d1, 0, [[K, CJ]]), in_=_ap(q, 0, [[K, CJ]]))
            nc.vector.memset(_ap(q, 0, [[K, CJ]]), 0.0)
            csz = FREE // SCAN_CHUNKS
            for ch in range(SCAN_CHUNKS):
                sl = slice(ch * csz, (ch + 1) * csz)
                nc.vector.tensor_tensor_scan(
                    out=R[:, sl], data0=q[:, sl], data1=d1[:, sl],
                    initial=0.0, op0=mult, op1=add,
                )
                nc.sync.dma_start(out=s0[:, sl], in_=R[:, sl])

            # ---- survivor pass ----------------------------------------------
            if cap:
                zeros = io.tile([P, ROWS], mybir.dt.float32)
                nc.gpsimd.memset(zeros[:], 0.0)
                off = 0
                for ti, sz in enumerate(surv_tiles):
                    sb = work.tile([P, L], mybir.dt.float32, name=f"sb{ti}")
                    nc.sync.dma_start(out=sb[:sz, :], in_=sc[off:off + sz, :])
                    nc.vector.tensor_scalar(
                        out=sb[:sz, 0:ROWS], in0=sb[:sz, 0:ROWS],
                        scalar1=1.0, scalar2=0.0, op0=amin, op1=amax,
                    )
                    rs = work.tile([P, ROWS], mybir.dt.float32, name=f"rs{ti}")
                    nc.vector.tensor_tensor_scan(
                        out=rs[:sz, :], data0=sb[:sz, 0:ROWS], data1=zeros[:sz, :],
                        initial=1.0, op0=mult, op1=add,
                    )
                    nc.sync.dma_start(
                        out=s1[off:off + sz, :], in_=rs[:sz, K:ROWS],
                    )
                    off += sz
    nc.compile()
    return nc


def get_nc(n_dd: int, surv_tiles: tuple):
    key = (n_dd, surv_tiles)
    if key not in _NC_CACHE:
        _NC_CACHE[key] = build_nc(n_dd, surv_tiles)
    return _NC_CACHE[key]


def _find_survivors(v: np.ndarray):
    """v: [1022] f32 (10*s).  Return j-indices with no exact-zero factor in
    rows m < K.  Factor zero <=> f32(v[j-1-m] + c_j) <= 0 (c = 1 - v), or,
    for the boundary rows (j <= m < K), c_j <= 0."""
    n = v.shape[0]
    c = (np.float32(1.0) - v).astype(np.float32)
    m = np.full(n, np.inf, dtype=np.float32)          # min of v over window
    if n > K:
        w = np.lib.stride_tricks.sliding_window_view(v, K).min(axis=1)
        m[K:] = w[:-1]                                # j >= K: v[j-K:j]
    run = np.minimum.accumulate(v)
    m[1:K] = run[:K - 1]                              # 0 < j < K: v[0:j]
    dead = (m + c).astype(np.float32) <= 0.0
    jk = np.arange(n) < K
    dead |= jk & (c <= 0.0)
    return np.nonzero(~dead)[0]


def prepare(score: np.ndarray, score_idx: np.ndarray):
    """Build (nc, in_maps, assemble) for the given inputs.  assemble(results)
    turns the per-core result dicts into the full output array."""
    score = np.asarray(score, dtype=np.float32)
    score_idx = np.asarray(score_idx)
    docs = score[score_idx]                  # [B, L]
    Bn, Ln = docs.shape
    assert Ln == L
    n_cores = 8
    dpc = Bn // n_cores                      # docs per core
    n_dd = dpc * 2
    assert n_dd == 8

    # per-core v arrays and survivor lists
    vs = []                                  # vs[core][dd] = v (f32 [1022])
    survs = []                               # survs[core] = list[(dd, j)]
    for cid in range(n_cores):
        vcore, scount = [], []
        for dl in range(dpc):
            s = docs[cid * dpc + dl, 1:-1].astype(np.float32)
            for t in range(2):
                sd = s if t == 0 else s[::-1]
                vcore.append((np.float32(10.0) * sd).astype(np.float32))
        slist = []
        for dd in range(n_dd):
            for j in _find_survivors(vcore[dd]):
                slist.append((dd, int(j)))
        vs.append(vcore)
        survs.append(slist)

    max_surv = max(len(s) for s in survs)
    tiles = []
    rem = max_surv
    while rem > 0:
        t = min(P, rem)
        if t < P:
            t = max(32, -(-t // 32) * 32)
        tiles.append(t)
        rem -= t
    surv_tiles = tuple(tiles)
    cap = sum(surv_tiles)

    in_maps = []
    for cid in range(n_cores):
        arr = np.zeros((n_dd, ARRW), np.float32)
        for dd in range(n_dd):
            arr[dd, K:K + N] = vs[cid][dd]
        im = {"arr": arr}
        if cap:
            scm = np.zeros((cap, L), np.float32)
            for slot, (dd, j) in enumerate(survs[cid]):
                v = vs[cid][dd]
                cj = np.float32(1.0) - v[j]
                hat = np.zeros(ROWS, np.float32)
                if j > 0:
                    hat[:j] = v[j - 1::-1]
                scm[slot, :ROWS] = (hat + cj).astype(np.float32)
            im["sc"] = scm
        in_maps.append(im)

    nc = get_nc(n_dd, surv_tiles)

    def assemble(results):
        full = np.zeros((Bn, 2, ROWS, N), np.float32)
        for cid in range(n_cores):
            r = results[cid]
            # prefix: [128, FREE] -> [dd, slot, j', m] -> [dd, m, col]
            pref = np.asarray(r["s0"]).reshape(n_dd, 16, CJ, K)
            pref = pref.transpose(0, 3, 1, 2).reshape(n_dd, K, NCOL)[:, :, :N]
            for dd in range(n_dd):
                doc, t = cid * dpc + dd // 2, dd % 2
                full[doc, t, :K, :] = pref[dd]
            if cap:
                s1v = np.asarray(r["s1"])
                for slot, (dd, j) in enumerate(survs[cid]):
                    doc, t = cid * dpc + dd // 2, dd % 2
                    full[doc, t, K:, j] = s1v[slot]
        return full

    return nc, in_maps, assemble


def kernel(score: np.ndarray, score_idx: np.ndarray) -> np.ndarray:
    nc, in_maps, assemble = prepare(score, score_idx)
    res = bass_utils.run_bass_kernel_spmd(nc, in_maps, core_ids=list(range(8)))
    return assemble(res.results)


# revision 12
# speedup vs baseline: 10.4924x; 1.4939x over previous
"""Trainium2 Bass kernel for nn_Gate_Net (Toeplitz + hard-sigmoid prob + cumprod gate).

Reference (per document row of 1024 scores):
  s = doc[1:-1]                                  # n = 1022
  hat[m, j] = s[j-1-m] if j-1-m >= 0 else 0      # [1021, 1022]
  p[m, j]  = clamp(10*(hat - s[j]) + 1, 0, 1)    # hard branch, res = 0.1
  fwd = cumprod(p, axis=0); bwd = same with s reversed
  out = stack([fwd, bwd]) per doc -> full [32, 2, 1021, 1022] f32

Key structure: with v = 10*s and c_j = 1 - v_j, factor(j, m) =
clamp(v[j-1-m] + c_j, 0, 1) (v[<0] := 0 reproduces the boundary rule).
A column's cumprod hits EXACT 0 at the first m with v[j-1-m] + c_j <= 0,
and everything below stays 0.  On real inputs ~99% of columns die within
the first K=128 rows, so:

  1. Prefix pass (device): rows 0..K-1 for all (padded) 1024 columns of
     all 8 doc-dirs at once.  Partition p = (dd, col-block-of-64); free
     axis t = j'*K + m.  q built from a shifted AP over a per-partition
     slice of v plus a broadcast c, clamped, then ONE segmented
     tensor_tensor_scan (scan: state = data0*state + data1; at each
     column start data0=0/data1=q0 resets the chain).  Result is DMAd
     with 128 contiguous 32 KiB descriptors -- no transpose needed; the
     host reorders (col-major -> row-major) on 4 MiB/core.
  2. Survivor pass (device): columns with no exact-zero factor among
     rows < K (found host-side with a sliding-window min; ~130/core)
     are scanned at full length col-major and the host scatters
     rows K.. into the output.
  3. Everything else is exactly 0 and is never written (host assembles
     into np.zeros).

Sharding: pure data parallel, 4 docs (8 doc-dirs) per core.
"""
import numpy as np

import concourse.bass as bass
import concourse.bacc as bacc
import concourse.tile as tile
from concourse import mybir
from concourse import bass_utils

P = 128            # SBUF partitions
L = 1024           # sentences per document
N = L - 2          # 1022 real columns per doc-dir
ROWS = N - 1       # 1021 output rows
K = 64             # dense prefix rows computed for every column
NCOL = 1024        # padded column count (cols N..NCOL-1 are garbage)
CJ = NCOL // 16    # 64 columns per partition slot
FREE = CJ * K      # 8192 free elems per partition in the prefix pass
ARRW = 2560        # [K zeros][1022 v][pad] at 0..1280, [1024 c][pad] at 1280..
COFF = 1280        # offset of the c region inside an arr row
SURV_ROWS = ROWS - K   # rows written per survivor column

_NC_CACHE: dict = {}


def _ap(t: bass.AP, delta: int, dims):
    """Custom free-dim AP over tile t (keeps t's partition pair)."""
    return bass.AP(tensor=t.tensor, offset=t.offset + delta,
                   ap=[list(t.ap[0])] + [list(d) for d in dims])


def build_nc(n_dd: int, surv_tiles: tuple):
    """Bass program: prefix pass for n_dd=8 doc-dirs + survivor scans."""
    assert n_dd == 8
    nc = bacc.Bacc("TRN2", target_bir_lowering=False, debug=False, num_devices=8)
    arr = nc.dram_tensor("arr", [n_dd, ARRW], mybir.dt.float32, kind="ExternalInput")
    cap = sum(surv_tiles)
    if cap:
        sc = nc.dram_tensor("sc", [cap, L], mybir.dt.float32, kind="ExternalInput")
        s1 = nc.dram_tensor("s1", [cap, SURV_ROWS], mybir.dt.float32,
                            kind="ExternalOutput")
    s0 = nc.dram_tensor("s0", [P, FREE], mybir.dt.float32, kind="ExternalOutput")

    add = mybir.AluOpType.add
    mult = mybir.AluOpType.mult
    amin = mybir.AluOpType.min
    amax = mybir.AluOpType.max

    with tile.TileContext(nc) as tc:
        with (
            tc.tile_pool(name="io", bufs=1) as io,
            tc.tile_pool(name="work", bufs=1) as work,
        ):
            # ---- prefix pass -------------------------------------------------
            # arr_sb[p, t] = v[J0 + t - K], J0 = (p % 16) * 64   (p = dd*16 + slot)
            arr_sb = io.tile([P, K + CJ], mybir.dt.float32)
            nc.sync.dma_start(
                out=arr_sb[:],
                in_=bass.AP(tensor=arr, offset=0,
                            ap=[[ARRW, 8], [CJ, 16], [1, K + CJ]]),
            )
            # c[p, j'] = 1 - v[J0 + j']  (host-precomputed, own region of arr)
            c_sb = io.tile([P, CJ], mybir.dt.float32)
            nc.sync.dma_start(
                out=c_sb[:],
                in_=bass.AP(tensor=arr, offset=COFF,
                            ap=[[ARRW, 8], [CJ, 16], [1, CJ]]),
            )
            q = work.tile([P, FREE], mybir.dt.float32)
            d1 = work.tile([P, FREE], mybir.dt.float32)
            R = work.tile([P, FREE], mybir.dt.float32)
            nc.gpsimd.memset(d1[:], 0.0)
            # q[p, j'*K + m] = v[J0 + j' - 1 - m] + c[J0 + j']
            nc.vector.tensor_tensor(
                out=_ap(q, 0, [[K, CJ], [1, K]]),
                in0=_ap(arr_sb, K - 1, [[1, CJ], [-1, K]]),
                in1=_ap(c_sb, 0, [[1, CJ], [0, K]]),
                op=add,
            )
            nc.vector.tensor_scalar(
                out=q[:], in0=q[:], scalar1=1.0, scalar2=0.0, op0=amin, op1=amax,
            )
            # segment resets at m == 0: d1 takes q's value, q becomes 0
            nc.gpsimd.tensor_copy(_ap(d1, 0, [[K, CJ]]), _ap(q, 0, [[K, CJ]]))
            nc.gpsimd.memset(_ap(q, 0, [[K, CJ]]), 0.0)
            half = FREE // 2
            for ch in range(2):
                sl = slice(ch * half, (ch + 1) * half)
                nc.vector.tensor_tensor_scan(
                    out=R[:, sl], data0=q[:, sl], data1=d1[:, sl],
                    initial=0.0, op0=mult, op1=add,
                )
                nc.sync.dma_start(out=s0[:, sl], in_=R[:, sl])

            # ---- survivor pass ----------------------------------------------
            if cap:
                zeros = io.tile([P, ROWS], mybir.dt.float32)
                nc.gpsimd.memset(zeros[:], 0.0)
                off = 0
                for ti, sz in enumerate(surv_tiles):
                    sb = work.tile([P, L], mybir.dt.float32, name=f"sb{ti}")
                    nc.sync.dma_start(out=sb[:sz, :], in_=sc[off:off + sz, :])
                    nc.gpsimd.tensor_scalar(
                        out=sb[:sz, 0:ROWS], in0=sb[:sz, 0:ROWS],
                        scalar1=1.0, scalar2=0.0, op0=amin, op1=amax,
                    )
                    rs = work.tile([P, ROWS], mybir.dt.float32, name=f"rs{ti}")
                    nc.vector.tensor_tensor_scan(
                        out=rs[:sz, :], data0=sb[:sz, 0:ROWS], data1=zeros[:sz, :],
                        initial=1.0, op0=mult, op1=add,
                    )
                    nc.sync.dma_start(
                        out=s1[off:off + sz, :], in_=rs[:sz, K:ROWS],
                    )
                    off += sz
    nc.compile()
    return nc


def get_nc(n_dd: int, surv_tiles: tuple):
    key = (n_dd, surv_tiles)
    if key not in _NC_CACHE:
        _NC_CACHE[key] = build_nc(n_dd, surv_tiles)
    return _NC_CACHE[key]


def _find_survivors(v: np.ndarray):
    """v: [1022] f32 (10*s).  Return j-indices with no exact-zero factor in
    rows m < K.  Factor zero <=> f32(v[j-1-m] + c_j) <= 0 (c = 1 - v), or,
    for the boundary rows (j <= m < K), c_j <= 0."""
    n = v.shape[0]
    c = (np.float32(1.0) - v).astype(np.float32)
    m = np.full(n, np.inf, dtype=np.float32)          # min of v over window
    if n > K:
        w = np.lib.stride_tricks.sliding_window_view(v, K).min(axis=1)
        m[K:] = w[:-1]                                # j >= K: v[j-K:j]
    run = np.minimum.accumulate(v)
    m[1:K] = run[:K - 1]                              # 0 < j < K: v[0:j]
    dead = (m + c).astype(np.float32) <= 0.0
    jk = np.arange(n) < K
    dead |= jk & (c <= 0.0)
    return np.nonzero(~dead)[0]


def prepare(score: np.ndarray, score_idx: np.ndarray):
    """Build (nc, in_maps, assemble) for the given inputs.  assemble(results)
    turns the per-core result dicts into the full output array."""
    score = np.asarray(score, dtype=np.float32)
    score_idx = np.asarray(score_idx)
    docs = score[score_idx]                  # [B, L]
    Bn, Ln = docs.shape
    assert Ln == L
    n_cores = 8
    dpc = Bn // n_cores                      # docs per core
    n_dd = dpc * 2
    assert n_dd == 8

    # per-core v arrays and survivor lists
    vs = []                                  # vs[core][dd] = v (f32 [1022])
    survs = []                               # survs[core] = list[(dd, j)]
    for cid in range(n_cores):
        vcore, scount = [], []
        for dl in range(dpc):
            s = docs[cid * dpc + dl, 1:-1].astype(np.float32)
            for t in range(2):
                sd = s if t == 0 else s[::-1]
                vcore.append((np.float32(10.0) * sd).astype(np.float32))
        slist = []
        for dd in range(n_dd):
            for j in _find_survivors(vcore[dd]):
                slist.append((dd, int(j)))
        vs.append(vcore)
        survs.append(slist)

    max_surv = max(len(s) for s in survs)
    tiles = []
    rem = max_surv
    while rem > 0:
        t = min(P, rem)
        if t < P:
            t = max(32, -(-t // 32) * 32)
        tiles.append(t)
        rem -= t
    surv_tiles = tuple(tiles)
    cap = sum(surv_tiles)

    in_maps = []
    for cid in range(n_cores):
        arr = np.zeros((n_dd, ARRW), np.float32)
        for dd in range(n_dd):
            v = vs[cid][dd]
            arr[dd, K:K + N] = v
            arr[dd, COFF:COFF + N] = (np.float32(1.0) - v).astype(np.float32)
        im = {"arr": arr}
        if cap:
            scm = np.zeros((cap, L), np.float32)
            for slot, (dd, j) in enumerate(survs[cid]):
                v = vs[cid][dd]
                cj = np.float32(1.0) - v[j]
                hat = np.zeros(ROWS, np.float32)
                if j > 0:
                    hat[:j] = v[j - 1::-1]
                scm[slot, :ROWS] = (hat + cj).astype(np.float32)
            im["sc"] = scm
        in_maps.append(im)

    nc = get_nc(n_dd, surv_tiles)

    def assemble(results):
        full = np.zeros((Bn, 2, ROWS, N), np.float32)
        for cid in range(n_cores):
            r = results[cid]
            # prefix: [128, FREE] -> [dd, slot, j', m] -> [dd, m, col]
            pref = np.asarray(r["s0"]).reshape(n_dd, 16, CJ, K)
            pref = pref.transpose(0, 3, 1, 2).reshape(n_dd, K, NCOL)[:, :, :N]
            for dd in range(n_dd):
                doc, t = cid * dpc + dd // 2, dd % 2
                full[doc, t, :K, :] = pref[dd]
            if cap:
                s1v = np.asarray(r["s1"])
                for slot, (dd, j) in enumerate(survs[cid]):
                    doc, t = cid * dpc + dd // 2, dd % 2
                    full[doc, t, K:, j] = s1v[slot]
        return full

    return nc, in_maps, assemble


def kernel(score: np.ndarray, score_idx: np.ndarray) -> np.ndarray:
    nc, in_maps, assemble = prepare(score, score_idx)
    res = bass_utils.run_bass_kernel_spmd(nc, in_maps, core_ids=list(range(8)))
    return assemble(res.results)


# revision 13
# speedup vs baseline: 10.9568x; 1.0443x over previous
"""Trainium2 Bass kernel for nn_Gate_Net (Toeplitz + hard-sigmoid prob + cumprod gate).

Reference (per document row of 1024 scores):
  s = doc[1:-1]                                  # n = 1022
  hat[m, j] = s[j-1-m] if j-1-m >= 0 else 0      # [1021, 1022]
  p[m, j]  = clamp(10*(hat - s[j]) + 1, 0, 1)    # hard branch, res = 0.1
  fwd = cumprod(p, axis=0); bwd = same with s reversed
  out = stack([fwd, bwd]) per doc -> full [32, 2, 1021, 1022] f32

Key structure: with v = 10*s and c_j = 1 - v_j, factor(j, m) =
clamp(v[j-1-m] + c_j, 0, 1) (v[<0] := 0 reproduces the boundary rule).
A column's cumprod hits EXACT 0 at the first m with v[j-1-m] + c_j <= 0,
and everything below stays 0.  On real inputs ~99% of columns die within
the first K=128 rows, so:

  1. Prefix pass (device): rows 0..K-1 for all (padded) 1024 columns of
     all 8 doc-dirs at once.  Partition p = (dd, col-block-of-64); free
     axis t = j'*K + m.  q built from a shifted AP over a per-partition
     slice of v plus a broadcast c, clamped, then ONE segmented
     tensor_tensor_scan (scan: state = data0*state + data1; at each
     column start data0=0/data1=q0 resets the chain).  Result is DMAd
     with 128 contiguous 32 KiB descriptors -- no transpose needed; the
     host reorders (col-major -> row-major) on 4 MiB/core.
  2. Survivor pass (device): columns with no exact-zero factor among
     rows < K (found host-side with a sliding-window min; ~130/core)
     are scanned at full length col-major and the host scatters
     rows K.. into the output.
  3. Everything else is exactly 0 and is never written (host assembles
     into np.zeros).

Sharding: pure data parallel, 4 docs (8 doc-dirs) per core.
"""
import numpy as np

import concourse.bass as bass
import concourse.bacc as bacc
import concourse.tile as tile
from concourse import mybir
from concourse import bass_utils

P = 128            # SBUF partitions
L = 1024           # sentences per document
N = L - 2          # 1022 real columns per doc-dir
ROWS = N - 1       # 1021 output rows
K = 64             # dense prefix rows computed for every column
NCOL = 1024        # padded column count (cols N..NCOL-1 are garbage)
CJ = NCOL // 16    # 64 columns per partition slot
FREE = CJ * K      # 8192 free elems per partition in the prefix pass
ARRW = 2560        # [K zeros][1022 v][pad] at 0..1280, [1024 c][pad] at 1280..
COFF = 1280        # offset of the c region inside an arr row
SURV_ROWS = ROWS - K   # rows written per survivor column

_NC_CACHE: dict = {}


def _ap(t: bass.AP, delta: int, dims):
    """Custom free-dim AP over tile t (keeps t's partition pair)."""
    return bass.AP(tensor=t.tensor, offset=t.offset + delta,
                   ap=[list(t.ap[0])] + [list(d) for d in dims])


def build_nc(n_dd: int, surv_tiles: tuple):
    """Bass program: prefix pass for n_dd=8 doc-dirs + survivor scans."""
    assert n_dd == 8
    nc = bacc.Bacc("TRN2", target_bir_lowering=False, debug=False, num_devices=8)
    arr = nc.dram_tensor("arr", [n_dd, ARRW], mybir.dt.float32, kind="ExternalInput")
    cap = sum(surv_tiles)
    if cap:
        sc = nc.dram_tensor("sc", [cap, L], mybir.dt.float32, kind="ExternalInput")
        s1 = nc.dram_tensor("s1", [cap, SURV_ROWS], mybir.dt.float32,
                            kind="ExternalOutput")
    s0 = nc.dram_tensor("s0", [P, FREE], mybir.dt.float32, kind="ExternalOutput")

    add = mybir.AluOpType.add
    mult = mybir.AluOpType.mult
    amin = mybir.AluOpType.min
    amax = mybir.AluOpType.max

    with tile.TileContext(nc) as tc:
        with (
            tc.tile_pool(name="io", bufs=1) as io,
            tc.tile_pool(name="work", bufs=1) as work,
        ):
            # ---- prefix pass -------------------------------------------------
            # arr_sb[p, t] = v[J0 + t - K], J0 = (p % 16) * 64   (p = dd*16 + slot)
            arr_sb = io.tile([P, K + CJ], mybir.dt.float32)
            nc.sync.dma_start(
                out=arr_sb[:],
                in_=bass.AP(tensor=arr, offset=0,
                            ap=[[ARRW, 8], [CJ, 16], [1, K + CJ]]),
            )
            # c[p, j'] = 1 - v[J0 + j']  (host-precomputed, own region of arr)
            c_sb = io.tile([P, CJ], mybir.dt.float32)
            nc.sync.dma_start(
                out=c_sb[:],
                in_=bass.AP(tensor=arr, offset=COFF,
                            ap=[[ARRW, 8], [CJ, 16], [1, CJ]]),
            )
            q = work.tile([P, FREE], mybir.dt.float32)
            d1 = work.tile([P, FREE], mybir.dt.float32)
            R = work.tile([P, FREE], mybir.dt.float32)
            nc.gpsimd.memset(d1[:], 0.0)
            # q[p, j'*K + m] = v[J0 + j' - 1 - m] + c[J0 + j']
            nc.vector.tensor_tensor(
                out=_ap(q, 0, [[K, CJ], [1, K]]),
                in0=_ap(arr_sb, K - 1, [[1, CJ], [-1, K]]),
                in1=_ap(c_sb, 0, [[1, CJ], [0, K]]),
                op=add,
            )
            nc.vector.tensor_scalar(
                out=q[:], in0=q[:], scalar1=1.0, scalar2=0.0, op0=amin, op1=amax,
            )
            # segment resets at m == 0: d1 takes q's value, q becomes 0
            nc.gpsimd.tensor_copy(_ap(d1, 0, [[K, CJ]]), _ap(q, 0, [[K, CJ]]))
            nc.gpsimd.memset(_ap(q, 0, [[K, CJ]]), 0.0)
            nchunk = 4
            csz = FREE // nchunk
            for ch in range(nchunk):
                sl = slice(ch * csz, (ch + 1) * csz)
                nc.vector.tensor_tensor_scan(
                    out=R[:, sl], data0=q[:, sl], data1=d1[:, sl],
                    initial=0.0, op0=mult, op1=add,
                )
                nc.sync.dma_start(out=s0[:, sl], in_=R[:, sl])

            # ---- survivor pass ----------------------------------------------
            if cap:
                zeros = io.tile([P, ROWS], mybir.dt.float32)
                nc.gpsimd.memset(zeros[:], 0.0)
                off = 0
                for ti, sz in enumerate(surv_tiles):
                    sb = work.tile([P, L], mybir.dt.float32, name=f"sb{ti}")
                    nc.sync.dma_start(out=sb[:sz, :], in_=sc[off:off + sz, :])
                    nc.gpsimd.tensor_scalar(
                        out=sb[:sz, 0:ROWS], in0=sb[:sz, 0:ROWS],
                        scalar1=1.0, scalar2=0.0, op0=amin, op1=amax,
                    )
                    rs = work.tile([P, ROWS], mybir.dt.float32, name=f"rs{ti}")
                    nc.vector.tensor_tensor_scan(
                        out=rs[:sz, :], data0=sb[:sz, 0:ROWS], data1=zeros[:sz, :],
                        initial=1.0, op0=mult, op1=add,
                    )
                    nc.sync.dma_start(
                        out=s1[off:off + sz, :], in_=rs[:sz, K:ROWS],
                    )
                    off += sz
    nc.compile()
    return nc


def get_nc(n_dd: int, surv_tiles: tuple):
    key = (n_dd, surv_tiles)
    if key not in _NC_CACHE:
        _NC_CACHE[key] = build_nc(n_dd, surv_tiles)
    return _NC_CACHE[key]


def _find_survivors(v: np.ndarray):
    """v: [1022] f32 (10*s).  Return j-indices with no exact-zero factor in
    rows m < K.  Factor zero <=> f32(v[j-1-m] + c_j) <= 0 (c = 1 - v), or,
    for the boundary rows (j <= m < K), c_j <= 0."""
    n = v.shape[0]
    c = (np.float32(1.0) - v).astype(np.float32)
    m = np.full(n, np.inf, dtype=np.float32)          # min of v over window
    if n > K:
        w = np.lib.stride_tricks.sliding_window_view(v, K).min(axis=1)
        m[K:] = w[:-1]                                # j >= K: v[j-K:j]
    run = np.minimum.accumulate(v)
    m[1:K] = run[:K - 1]                              # 0 < j < K: v[0:j]
    dead = (m + c).astype(np.float32) <= 0.0
    jk = np.arange(n) < K
    dead |= jk & (c <= 0.0)
    return np.nonzero(~dead)[0]


def prepare(score: np.ndarray, score_idx: np.ndarray):
    """Build (nc, in_maps, assemble) for the given inputs.  assemble(results)
    turns the per-core result dicts into the full output array."""
    score = np.asarray(score, dtype=np.float32)
    score_idx = np.asarray(score_idx)
    docs = score[score_idx]                  # [B, L]
    Bn, Ln = docs.shape
    assert Ln == L
    n_cores = 8
    dpc = Bn // n_cores                      # docs per core
    n_dd = dpc * 2
    assert n_dd == 8

    # per-core v arrays and survivor lists
    vs = []                                  # vs[core][dd] = v (f32 [1022])
    survs = []                               # survs[core] = list[(dd, j)]
    for cid in range(n_cores):
        vcore, scount = [], []
        for dl in range(dpc):
            s = docs[cid * dpc + dl, 1:-1].astype(np.float32)
            for t in range(2):
                sd = s if t == 0 else s[::-1]
                vcore.append((np.float32(10.0) * sd).astype(np.float32))
        slist = []
        for dd in range(n_dd):
            for j in _find_survivors(vcore[dd]):
                slist.append((dd, int(j)))
        vs.append(vcore)
        survs.append(slist)

    max_surv = max(len(s) for s in survs)
    tiles = []
    rem = max_surv
    while rem > 0:
        t = min(P, rem)
        if t < P:
            t = max(32, -(-t // 32) * 32)
        tiles.append(t)
        rem -= t
    surv_tiles = tuple(tiles)
    cap = sum(surv_tiles)

    in_maps = []
    for cid in range(n_cores):
        arr = np.zeros((n_dd, ARRW), np.float32)
        for dd in range(n_dd):
            v = vs[cid][dd]
            arr[dd, K:K + N] = v
            arr[dd, COFF:COFF + N] = (np.float32(1.0) - v).astype(np.float32)
        im = {"arr": arr}
        if cap:
            scm = np.zeros((cap, L), np.float32)
            for slot, (dd, j) in enumerate(survs[cid]):
                v = vs[cid][dd]
                cj = np.float32(1.0) - v[j]
                hat = np.zeros(ROWS, np.float32)
                if j > 0:
                    hat[:j] = v[j - 1::-1]
                scm[slot, :ROWS] = (hat + cj).astype(np.float32)
            im["sc"] = scm
        in_maps.append(im)

    nc = get_nc(n_dd, surv_tiles)

    def assemble(results):
        full = np.zeros((Bn, 2, ROWS, N), np.float32)
        for cid in range(n_cores):
            r = results[cid]
            # prefix: [128, FREE] -> [dd, slot, j', m] -> [dd, m, col]
            pref = np.asarray(r["s0"]).reshape(n_dd, 16, CJ, K)
            pref = pref.transpose(0, 3, 1, 2).reshape(n_dd, K, NCOL)[:, :, :N]
            for dd in range(n_dd):
                doc, t = cid * dpc + dd // 2, dd % 2
                full[doc, t, :K, :] = pref[dd]
            if cap:
                s1v = np.asarray(r["s1"])
                for slot, (dd, j) in enumerate(survs[cid]):
                    doc, t = cid * dpc + dd // 2, dd % 2
                    full[doc, t, K:, j] = s1v[slot]
        return full

    return nc, in_maps, assemble


def kernel(score: np.ndarray, score_idx: np.ndarray) -> np.ndarray:
    nc, in_maps, assemble = prepare(score, score_idx)
    res = bass_utils.run_bass_kernel_spmd(nc, in_maps, core_ids=list(range(8)))
    return assemble(res.results)


# revision 15
# speedup vs baseline: 11.3054x; 1.0318x over previous
"""Trainium2 Bass kernel for nn_Gate_Net (Toeplitz + hard-sigmoid prob + cumprod gate).

Reference (per document row of 1024 scores):
  s = doc[1:-1]                                  # n = 1022
  hat[m, j] = s[j-1-m] if j-1-m >= 0 else 0      # [1021, 1022]
  p[m, j]  = clamp(10*(hat - s[j]) + 1, 0, 1)    # hard branch, res = 0.1
  fwd = cumprod(p, axis=0); bwd = same with s reversed
  out = stack([fwd, bwd]) per doc -> full [32, 2, 1021, 1022] f32

Key structure: with v = 10*s and c_j = 1 - v_j, factor(j, m) =
clamp(v[j-1-m] + c_j, 0, 1) (v[<0] := 0 reproduces the boundary rule).
A column's cumprod hits EXACT 0 at the first m with v[j-1-m] + c_j <= 0,
and everything below stays 0.  On real inputs ~99% of columns die within
the first K=128 rows, so:

  1. Prefix pass (device): rows 0..K-1 for all (padded) 1024 columns of
     all 8 doc-dirs at once.  Partition p = (dd, col-block-of-64); free
     axis t = j'*K + m.  q built from a shifted AP over a per-partition
     slice of v plus a broadcast c, clamped, then ONE segmented
     tensor_tensor_scan (scan: state = data0*state + data1; at each
     column start data0=0/data1=q0 resets the chain).  Result is DMAd
     with 128 contiguous 32 KiB descriptors -- no transpose needed; the
     host reorders (col-major -> row-major) on 4 MiB/core.
  2. Survivor pass (device): columns with no exact-zero factor among
     rows < K (found host-side with a sliding-window min; ~130/core)
     are scanned at full length col-major and the host scatters
     rows K.. into the output.
  3. Everything else is exactly 0 and is never written (host assembles
     into np.zeros).

Sharding: pure data parallel, 4 docs (8 doc-dirs) per core.
"""
import numpy as np

import concourse.bass as bass
import concourse.bacc as bacc
import concourse.tile as tile
from concourse import mybir
from concourse import bass_utils

P = 128            # SBUF partitions
L = 1024           # sentences per document
N = L - 2          # 1022 real columns per doc-dir
ROWS = N - 1       # 1021 output rows
K = 48             # dense prefix rows computed for every column
NCOL = 1024        # padded column count (cols N..NCOL-1 are garbage)
CJ = NCOL // 16    # 64 columns per partition slot
FREE = CJ * K      # 8192 free elems per partition in the prefix pass
ARRW = 2560        # [K zeros][1022 v][pad] at 0..1280, [1024 c][pad] at 1280..
COFF = 1280        # offset of the c region inside an arr row
SURV_ROWS = ROWS - K   # rows written per survivor column

_NC_CACHE: dict = {}


def _ap(t: bass.AP, delta: int, dims):
    """Custom free-dim AP over tile t (keeps t's partition pair)."""
    return bass.AP(tensor=t.tensor, offset=t.offset + delta,
                   ap=[list(t.ap[0])] + [list(d) for d in dims])


def build_nc(n_dd: int, surv_tiles: tuple):
    """Bass program: prefix pass for n_dd=8 doc-dirs + survivor scans."""
    assert n_dd == 8
    nc = bacc.Bacc("TRN2", target_bir_lowering=False, debug=False, num_devices=8)
    arr = nc.dram_tensor("arr", [n_dd, ARRW], mybir.dt.float32, kind="ExternalInput")
    cap = sum(surv_tiles)
    if cap:
        sc = nc.dram_tensor("sc", [cap, L], mybir.dt.float32, kind="ExternalInput")
        s1 = nc.dram_tensor("s1", [cap, SURV_ROWS], mybir.dt.float32,
                            kind="ExternalOutput")
    s0 = nc.dram_tensor("s0", [P, FREE], mybir.dt.float32, kind="ExternalOutput")

    add = mybir.AluOpType.add
    mult = mybir.AluOpType.mult
    amin = mybir.AluOpType.min
    amax = mybir.AluOpType.max

    with tile.TileContext(nc) as tc:
        with (
            tc.tile_pool(name="io", bufs=1) as io,
            tc.tile_pool(name="work", bufs=1) as work,
        ):
            # ---- prefix pass -------------------------------------------------
            # arr_sb[p, t] = v[J0 + t - K], J0 = (p % 16) * 64   (p = dd*16 + slot)
            # Input loads go through gpsimd (SWDGE) whose queue starts ~1.5us
            # earlier than sync's in the NEFF preamble.
            arr_sb = io.tile([P, K + CJ], mybir.dt.float32)
            nc.gpsimd.dma_start(
                out=arr_sb[:],
                in_=bass.AP(tensor=arr, offset=0,
                            ap=[[ARRW, 8], [CJ, 16], [1, K + CJ]]),
            )
            # c[p, j'] = 1 - v[J0 + j']  (host-precomputed, own region of arr)
            c_sb = io.tile([P, CJ], mybir.dt.float32)
            nc.gpsimd.dma_start(
                out=c_sb[:],
                in_=bass.AP(tensor=arr, offset=COFF,
                            ap=[[ARRW, 8], [CJ, 16], [1, CJ]]),
            )
            # survivor inputs early on the sync queue
            zeros = None
            sbs = []
            if cap:
                zeros = io.tile([P, ROWS], mybir.dt.float32)
                off = 0
                for ti, sz in enumerate(surv_tiles):
                    sb = work.tile([P, L], mybir.dt.float32, name=f"sb{ti}")
                    nc.sync.dma_start(out=sb[:sz, :], in_=sc[off:off + sz, :])
                    sbs.append(sb)
                    off += sz

            q = work.tile([P, FREE], mybir.dt.float32)
            d1 = work.tile([P, FREE], mybir.dt.float32)
            R = work.tile([P, FREE], mybir.dt.float32)
            nchunk = 4
            csz = FREE // nchunk
            JV = CJ - CJ // nchunk          # j'-slots computed on vector
            # q[p, j'*K + m] = v[J0 + j' - 1 - m] + c[J0 + j']  (split V/G)
            def q_build(eng, j0, j1):
                n = j1 - j0
                eng.tensor_tensor(
                    out=_ap(q, j0 * K, [[K, n], [1, K]]),
                    in0=_ap(arr_sb, K - 1 + j0, [[1, n], [-1, K]]),
                    in1=_ap(c_sb, j0, [[1, n], [0, K]]),
                    op=add,
                )
                eng.tensor_scalar(
                    out=q[:, j0 * K:j1 * K], in0=q[:, j0 * K:j1 * K],
                    scalar1=1.0, scalar2=0.0, op0=amin, op1=amax,
                )
                # segment resets at m == 0: d1 takes q's value, q becomes 0
                eng.tensor_copy(_ap(d1, j0 * K, [[K, n]]), _ap(q, j0 * K, [[K, n]]))
                eng.memset(_ap(q, j0 * K, [[K, n]]), 0.0)

            # gpsimd: d1 zeros, then its q share + survivor clamps interleaved
            nc.gpsimd.memset(d1[:], 0.0)
            if cap:
                nc.gpsimd.memset(zeros[:], 0.0)
                sz0 = surv_tiles[0]
                nc.gpsimd.tensor_scalar(
                    out=sbs[0][:sz0, 0:ROWS], in0=sbs[0][:sz0, 0:ROWS],
                    scalar1=1.0, scalar2=0.0, op0=amin, op1=amax,
                )
            q_build(nc.gpsimd, JV, CJ)
            if cap:
                off = surv_tiles[0]
                for ti, sz in list(enumerate(surv_tiles))[1:]:
                    nc.gpsimd.tensor_scalar(
                        out=sbs[ti][:sz, 0:ROWS], in0=sbs[ti][:sz, 0:ROWS],
                        scalar1=1.0, scalar2=0.0, op0=amin, op1=amax,
                    )

            # vector: its q share, then scans with survivor scans interleaved
            q_build(nc.vector, 0, JV)

            def svscan(ti, off):
                sz = surv_tiles[ti]
                rs = work.tile([P, ROWS], mybir.dt.float32, name=f"rs{ti}")
                nc.vector.tensor_tensor_scan(
                    out=rs[:sz, :], data0=sbs[ti][:sz, 0:ROWS],
                    data1=zeros[:sz, :], initial=1.0, op0=mult, op1=add,
                )
                nc.sync.dma_start(out=s1[off:off + sz, :], in_=rs[:sz, K:ROWS])

            for ch in range(nchunk - 1):
                sl = slice(ch * csz, (ch + 1) * csz)
                nc.vector.tensor_tensor_scan(
                    out=R[:, sl], data0=q[:, sl], data1=d1[:, sl],
                    initial=0.0, op0=mult, op1=add,
                )
                nc.sync.dma_start(out=s0[:, sl], in_=R[:, sl])
            if cap:
                svscan(0, 0)
            sl = slice((nchunk - 1) * csz, FREE)
            nc.vector.tensor_tensor_scan(
                out=R[:, sl], data0=q[:, sl], data1=d1[:, sl],
                initial=0.0, op0=mult, op1=add,
            )
            nc.sync.dma_start(out=s0[:, sl], in_=R[:, sl])
            if cap:
                off = surv_tiles[0]
                for ti, sz in list(enumerate(surv_tiles))[1:]:
                    svscan(ti, off)
                    off += sz
    nc.compile()
    return nc


def get_nc(n_dd: int, surv_tiles: tuple):
    key = (n_dd, surv_tiles)
    if key not in _NC_CACHE:
        _NC_CACHE[key] = build_nc(n_dd, surv_tiles)
    return _NC_CACHE[key]


def _find_survivors(v: np.ndarray):
    """v: [1022] f32 (10*s).  Return j-indices with no exact-zero factor in
    rows m < K.  Factor zero <=> f32(v[j-1-m] + c_j) <= 0 (c = 1 - v), or,
    for the boundary rows (j <= m < K), c_j <= 0."""
    n = v.shape[0]
    c = (np.float32(1.0) - v).astype(np.float32)
    m = np.full(n, np.inf, dtype=np.float32)          # min of v over window
    if n > K:
        w = np.lib.stride_tricks.sliding_window_view(v, K).min(axis=1)
        m[K:] = w[:-1]                                # j >= K: v[j-K:j]
    run = np.minimum.accumulate(v)
    m[1:K] = run[:K - 1]                              # 0 < j < K: v[0:j]
    dead = (m + c).astype(np.float32) <= 0.0
    jk = np.arange(n) < K
    dead |= jk & (c <= 0.0)
    return np.nonzero(~dead)[0]


def prepare(score: np.ndarray, score_idx: np.ndarray):
    """Build (nc, in_maps, assemble) for the given inputs.  assemble(results)
    turns the per-core result dicts into the full output array."""
    score = np.asarray(score, dtype=np.float32)
    score_idx = np.asarray(score_idx)
    docs = score[score_idx]                  # [B, L]
    Bn, Ln = docs.shape
    assert Ln == L
    n_cores = 8
    dpc = Bn // n_cores                      # docs per core
    n_dd = dpc * 2
    assert n_dd == 8

    # per-core v arrays and survivor lists
    vs = []                                  # vs[core][dd] = v (f32 [1022])
    survs = []                               # survs[core] = list[(dd, j)]
    for cid in range(n_cores):
        vcore, scount = [], []
        for dl in range(dpc):
            s = docs[cid * dpc + dl, 1:-1].astype(np.float32)
            for t in range(2):
                sd = s if t == 0 else s[::-1]
                vcore.append((np.float32(10.0) * sd).astype(np.float32))
        slist = []
        for dd in range(n_dd):
            for j in _find_survivors(vcore[dd]):
                slist.append((dd, int(j)))
        vs.append(vcore)
        survs.append(slist)

    max_surv = max(len(s) for s in survs)
    tiles = []
    rem = max_surv
    while rem > 0:
        t = min(P, rem)
        if t < P:
            t = max(32, -(-t // 32) * 32)
        tiles.append(t)
        rem -= t
    surv_tiles = tuple(tiles)
    cap = sum(surv_tiles)

    in_maps = []
    for cid in range(n_cores):
        arr = np.zeros((n_dd, ARRW), np.float32)
        for dd in range(n_dd):
            v = vs[cid][dd]
            arr[dd, K:K + N] = v
            arr[dd, COFF:COFF + N] = (np.float32(1.0) - v).astype(np.float32)
        im = {"arr": arr}
        if cap:
            scm = np.zeros((cap, L), np.float32)
            for slot, (dd, j) in enumerate(survs[cid]):
                v = vs[cid][dd]
                cj = np.float32(1.0) - v[j]
                hat = np.zeros(ROWS, np.float32)
                if j > 0:
                    hat[:j] = v[j - 1::-1]
                scm[slot, :ROWS] = (hat + cj).astype(np.float32)
            im["sc"] = scm
        in_maps.append(im)

    nc = get_nc(n_dd, surv_tiles)

    def assemble(results):
        full = np.zeros((Bn, 2, ROWS, N), np.float32)
        for cid in range(n_cores):
            r = results[cid]
            # prefix: [128, FREE] -> [dd, slot, j', m] -> [dd, m, col]
            pref = np.asarray(r["s0"]).reshape(n_dd, 16, CJ, K)
            pref = pref.transpose(0, 3, 1, 2).reshape(n_dd, K, NCOL)[:, :, :N]
            for dd in range(n_dd):
                doc, t = cid * dpc + dd // 2, dd % 2
                full[doc, t, :K, :] = pref[dd]
            if cap:
                s1v = np.asarray(r["s1"])
                for slot, (dd, j) in enumerate(survs[cid]):
                    doc, t = cid * dpc + dd // 2, dd % 2
                    full[doc, t, K:, j] = s1v[slot]
        return full

    return nc, in_maps, assemble


def kernel(score: np.ndarray, score_idx: np.ndarray) -> np.ndarray:
    nc, in_maps, assemble = prepare(score, score_idx)
    res = bass_utils.run_bass_kernel_spmd(nc, in_maps, core_ids=list(range(8)))
    return assemble(res.results)


# revision 16
# speedup vs baseline: 12.1666x; 1.0762x over previous
"""Trainium2 Bass kernel for nn_Gate_Net (Toeplitz + hard-sigmoid prob + cumprod gate).

Reference (per document row of 1024 scores):
  s = doc[1:-1]                                  # n = 1022
  hat[m, j] = s[j-1-m] if j-1-m >= 0 else 0      # [1021, 1022]
  p[m, j]  = clamp(10*(hat - s[j]) + 1, 0, 1)    # hard branch, res = 0.1
  fwd = cumprod(p, axis=0); bwd = same with s reversed
  out = stack([fwd, bwd]) per doc -> full [32, 2, 1021, 1022] f32

Key structure: with v = 10*s and c_j = 1 - v_j, factor(j, m) =
clamp(v[j-1-m] + c_j, 0, 1) (v[<0] := 0 reproduces the boundary rule).
A column's cumprod hits EXACT 0 at the first m with v[j-1-m] + c_j <= 0,
and everything below stays 0.  On real inputs ~99% of columns die within
the first K=128 rows, so:

  1. Prefix pass (device): rows 0..K-1 for all (padded) 1024 columns of
     all 8 doc-dirs at once.  Partition p = (dd, col-block-of-64); free
     axis t = j'*K + m.  q built from a shifted AP over a per-partition
     slice of v plus a broadcast c, clamped, then ONE segmented
     tensor_tensor_scan (scan: state = data0*state + data1; at each
     column start data0=0/data1=q0 resets the chain).  Result is DMAd
     with 128 contiguous 32 KiB descriptors -- no transpose needed; the
     host reorders (col-major -> row-major) on 4 MiB/core.
  2. Survivor pass (device): columns with no exact-zero factor among
     rows < K (found host-side with a sliding-window min; ~130/core)
     are scanned at full length col-major and the host scatters
     rows K.. into the output.
  3. Everything else is exactly 0 and is never written (host assembles
     into np.zeros).

Sharding: pure data parallel, 4 docs (8 doc-dirs) per core.
"""
import numpy as np

import concourse.bass as bass
import concourse.bacc as bacc
import concourse.tile as tile
from concourse import mybir
from concourse import bass_utils

P = 128            # SBUF partitions
L = 1024           # sentences per document
N = L - 2          # 1022 real columns per doc-dir
ROWS = N - 1       # 1021 output rows
K = 48             # dense prefix rows computed for every column
NCOL = 1024        # padded column count (cols N..NCOL-1 are garbage)
CJ = NCOL // 16    # 64 columns per partition slot
FREE = CJ * K      # 8192 free elems per partition in the prefix pass
ARRW = 2560        # [K zeros][1022 v][pad] at 0..1280, [1024 c][pad] at 1280..
COFF = 1280        # offset of the c region inside an arr row
SURV_ROWS = ROWS - K   # rows written per survivor column

_NC_CACHE: dict = {}


def _ap(t: bass.AP, delta: int, dims):
    """Custom free-dim AP over tile t (keeps t's partition pair)."""
    return bass.AP(tensor=t.tensor, offset=t.offset + delta,
                   ap=[list(t.ap[0])] + [list(d) for d in dims])


def build_nc(n_dd: int, surv_tiles: tuple):
    """Bass program: prefix pass for n_dd=8 doc-dirs + survivor scans."""
    assert n_dd == 8
    nc = bacc.Bacc("TRN2", target_bir_lowering=False, debug=False, num_devices=8)
    arr = nc.dram_tensor("arr", [n_dd, ARRW], mybir.dt.float32, kind="ExternalInput")
    cap = sum(surv_tiles)
    if cap:
        sc = nc.dram_tensor("sc", [cap, L], mybir.dt.float32, kind="ExternalInput")
        s1 = nc.dram_tensor("s1", [cap, SURV_ROWS], mybir.dt.float32,
                            kind="ExternalOutput")
    s0 = nc.dram_tensor("s0", [P, FREE], mybir.dt.float32, kind="ExternalOutput")

    add = mybir.AluOpType.add
    mult = mybir.AluOpType.mult
    amin = mybir.AluOpType.min
    amax = mybir.AluOpType.max

    with tile.TileContext(nc) as tc:
        with (
            tc.tile_pool(name="io", bufs=1) as io,
            tc.tile_pool(name="work", bufs=1) as work,
        ):
            # ---- prefix pass -------------------------------------------------
            # arr_sb[p, t] = v[J0 + t - K], J0 = (p % 16) * 64   (p = dd*16 + slot)
            arr_sb = io.tile([P, K + CJ], mybir.dt.float32)
            nc.sync.dma_start(
                out=arr_sb[:],
                in_=bass.AP(tensor=arr, offset=0,
                            ap=[[ARRW, 8], [CJ, 16], [1, K + CJ]]),
            )
            # c[p, j'] = 1 - v[J0 + j']  (host-precomputed, own region of arr)
            c_sb = io.tile([P, CJ], mybir.dt.float32)
            nc.sync.dma_start(
                out=c_sb[:],
                in_=bass.AP(tensor=arr, offset=COFF,
                            ap=[[ARRW, 8], [CJ, 16], [1, CJ]]),
            )
            # survivor inputs early on the sync queue
            zeros = None
            sbs = []
            if cap:
                zeros = io.tile([P, ROWS], mybir.dt.float32)
                off = 0
                for ti, sz in enumerate(surv_tiles):
                    sb = work.tile([P, L], mybir.dt.float32, name=f"sb{ti}")
                    nc.sync.dma_start(out=sb[:sz, :], in_=sc[off:off + sz, :])
                    sbs.append(sb)
                    off += sz

            q = work.tile([P, FREE], mybir.dt.float32)
            d1 = work.tile([P, FREE], mybir.dt.float32)
            R = work.tile([P, FREE], mybir.dt.float32)
            nchunk = 4
            csz = FREE // nchunk
            JV = CJ - CJ // nchunk          # j'-slots computed on vector
            # q[p, j'*K + m] = v[J0 + j' - 1 - m] + c[J0 + j']  (split V/G)
            def q_build(eng, j0, j1):
                n = j1 - j0
                eng.tensor_tensor(
                    out=_ap(q, j0 * K, [[K, n], [1, K]]),
                    in0=_ap(arr_sb, K - 1 + j0, [[1, n], [-1, K]]),
                    in1=_ap(c_sb, j0, [[1, n], [0, K]]),
                    op=add,
                )
                eng.tensor_scalar(
                    out=q[:, j0 * K:j1 * K], in0=q[:, j0 * K:j1 * K],
                    scalar1=1.0, scalar2=0.0, op0=amin, op1=amax,
                )
                # segment resets at m == 0: d1 takes q's value, q becomes 0
                eng.tensor_copy(_ap(d1, j0 * K, [[K, n]]), _ap(q, j0 * K, [[K, n]]))
                eng.memset(_ap(q, j0 * K, [[K, n]]), 0.0)

            # gpsimd: d1 zeros, then its q share + survivor clamps interleaved
            nc.gpsimd.memset(d1[:], 0.0)
            if cap:
                nc.gpsimd.memset(zeros[:], 0.0)
                sz0 = surv_tiles[0]
                nc.gpsimd.tensor_scalar(
                    out=sbs[0][:sz0, 0:ROWS], in0=sbs[0][:sz0, 0:ROWS],
                    scalar1=1.0, scalar2=0.0, op0=amin, op1=amax,
                )
            q_build(nc.gpsimd, JV, CJ)
            if cap:
                off = surv_tiles[0]
                for ti, sz in list(enumerate(surv_tiles))[1:]:
                    nc.gpsimd.tensor_scalar(
                        out=sbs[ti][:sz, 0:ROWS], in0=sbs[ti][:sz, 0:ROWS],
                        scalar1=1.0, scalar2=0.0, op0=amin, op1=amax,
                    )

            # vector: its q share, then scans with survivor scans interleaved
            q_build(nc.vector, 0, JV)

            def svscan(ti, off):
                sz = surv_tiles[ti]
                rs = work.tile([P, ROWS], mybir.dt.float32, name=f"rs{ti}")
                nc.vector.tensor_tensor_scan(
                    out=rs[:sz, :], data0=sbs[ti][:sz, 0:ROWS],
                    data1=zeros[:sz, :], initial=1.0, op0=mult, op1=add,
                )
                nc.sync.dma_start(out=s1[off:off + sz, :], in_=rs[:sz, K:ROWS])

            for ch in range(nchunk - 1):
                sl = slice(ch * csz, (ch + 1) * csz)
                nc.vector.tensor_tensor_scan(
                    out=R[:, sl], data0=q[:, sl], data1=d1[:, sl],
                    initial=0.0, op0=mult, op1=add,
                )
                nc.sync.dma_start(out=s0[:, sl], in_=R[:, sl])
            if cap:
                svscan(0, 0)
            sl = slice((nchunk - 1) * csz, FREE)
            nc.vector.tensor_tensor_scan(
                out=R[:, sl], data0=q[:, sl], data1=d1[:, sl],
                initial=0.0, op0=mult, op1=add,
            )
            nc.sync.dma_start(out=s0[:, sl], in_=R[:, sl])
            if cap:
                off = surv_tiles[0]
                for ti, sz in list(enumerate(surv_tiles))[1:]:
                    svscan(ti, off)
                    off += sz
    nc.compile()
    return nc


def get_nc(n_dd: int, surv_tiles: tuple):
    key = (n_dd, surv_tiles)
    if key not in _NC_CACHE:
        _NC_CACHE[key] = build_nc(n_dd, surv_tiles)
    return _NC_CACHE[key]


def _find_survivors(v: np.ndarray):
    """v: [1022] f32 (10*s).  Return j-indices with no exact-zero factor in
    rows m < K.  Factor zero <=> f32(v[j-1-m] + c_j) <= 0 (c = 1 - v), or,
    for the boundary rows (j <= m < K), c_j <= 0."""
    n = v.shape[0]
    c = (np.float32(1.0) - v).astype(np.float32)
    m = np.full(n, np.inf, dtype=np.float32)          # min of v over window
    if n > K:
        w = np.lib.stride_tricks.sliding_window_view(v, K).min(axis=1)
        m[K:] = w[:-1]                                # j >= K: v[j-K:j]
    run = np.minimum.accumulate(v)
    m[1:K] = run[:K - 1]                              # 0 < j < K: v[0:j]
    dead = (m + c).astype(np.float32) <= 0.0
    jk = np.arange(n) < K
    dead |= jk & (c <= 0.0)
    return np.nonzero(~dead)[0]


def prepare(score: np.ndarray, score_idx: np.ndarray):
    """Build (nc, in_maps, assemble) for the given inputs.  assemble(results)
    turns the per-core result dicts into the full output array."""
    score = np.asarray(score, dtype=np.float32)
    score_idx = np.asarray(score_idx)
    docs = score[score_idx]                  # [B, L]
    Bn, Ln = docs.shape
    assert Ln == L
    n_cores = 8
    dpc = Bn // n_cores                      # docs per core
    n_dd = dpc * 2
    assert n_dd == 8

    # per-core v arrays and survivor lists
    vs = []                                  # vs[core][dd] = v (f32 [1022])
    survs = []                               # survs[core] = list[(dd, j)]
    for cid in range(n_cores):
        vcore, scount = [], []
        for dl in range(dpc):
            s = docs[cid * dpc + dl, 1:-1].astype(np.float32)
            for t in range(2):
                sd = s if t == 0 else s[::-1]
                vcore.append((np.float32(10.0) * sd).astype(np.float32))
        slist = []
        for dd in range(n_dd):
            for j in _find_survivors(vcore[dd]):
                slist.append((dd, int(j)))
        vs.append(vcore)
        survs.append(slist)

    max_surv = max(len(s) for s in survs)
    tiles = []
    rem = max_surv
    while rem > 0:
        t = min(P, rem)
        if t < P:
            t = max(32, -(-t // 32) * 32)
        tiles.append(t)
        rem -= t
    surv_tiles = tuple(tiles)
    cap = sum(surv_tiles)

    in_maps = []
    for cid in range(n_cores):
        arr = np.zeros((n_dd, ARRW), np.float32)
        for dd in range(n_dd):
            v = vs[cid][dd]
            arr[dd, K:K + N] = v
            arr[dd, COFF:COFF + N] = (np.float32(1.0) - v).astype(np.float32)
        im = {"arr": arr}
        if cap:
            scm = np.zeros((cap, L), np.float32)
            for slot, (dd, j) in enumerate(survs[cid]):
                v = vs[cid][dd]
                cj = np.float32(1.0) - v[j]
                hat = np.zeros(ROWS, np.float32)
                if j > 0:
                    hat[:j] = v[j - 1::-1]
                scm[slot, :ROWS] = (hat + cj).astype(np.float32)
            im["sc"] = scm
        in_maps.append(im)

    nc = get_nc(n_dd, surv_tiles)

    def assemble(results):
        full = np.zeros((Bn, 2, ROWS, N), np.float32)
        for cid in range(n_cores):
            r = results[cid]
            # prefix: [128, FREE] -> [dd, slot, j', m] -> [dd, m, col]
            pref = np.asarray(r["s0"]).reshape(n_dd, 16, CJ, K)
            pref = pref.transpose(0, 3, 1, 2).reshape(n_dd, K, NCOL)[:, :, :N]
            for dd in range(n_dd):
                doc, t = cid * dpc + dd // 2, dd % 2
                full[doc, t, :K, :] = pref[dd]
            if cap:
                s1v = np.asarray(r["s1"])
                for slot, (dd, j) in enumerate(survs[cid]):
                    doc, t = cid * dpc + dd // 2, dd % 2
                    full[doc, t, K:, j] = s1v[slot]
        return full

    return nc, in_maps, assemble


def kernel(score: np.ndarray, score_idx: np.ndarray) -> np.ndarray:
    nc, in_maps, assemble = prepare(score, score_idx)
    res = bass_utils.run_bass_kernel_spmd(nc, in_maps, core_ids=list(range(8)))
    return assemble(res.results)
